# revision 1
# baseline (speedup 1.0000x reference)
"""Trainium2 Bass kernel for CustomConv1d.

Problem: y = conv1d(x, weight, bias), x [32, 256, 4096] f32,
weight [256, 256, 5] f32, bias [256] f32, stride 1, pad 2.

Strategy: data-parallel over batch across 8 NeuronCores (4 batches/core,
weights+bias broadcast, no collectives). Per core the conv is computed as
matmuls on the tensor engine: for each output-channel chunk (128) and each
512-wide output tile, accumulate 10 matmuls in PSUM (5 taps x 2 input-channel
chunks of 128):

  out[co, w] = sum_{k, ci} weight[co, ci, k] * xpad[ci, w + k]

with lhsT = weight slice [ci(128 part), co(128)] and rhs = x slice
[ci(128 part), 512], both tagged float32r (full-rate fp32 matmul, ~1e-4
rel err). x arrives host-padded ([.., W+4]) so every tap is a plain
contiguous slice and no memset is needed (memset can't write f32r).
"""

import os

import numpy as np

import concourse.mybir as mybir
import concourse.tile as tile
from concourse import bacc
from concourse.bass_utils import run_bass_kernel_spmd


BF16 = mybir.dt.bfloat16

B, CIN, COUT, W, K, PAD = 32, 256, 256, 4096, 5, 2
NCORES = 8
BPC = B // NCORES          # batches per core
P = 128                    # partition dim
NT = 512                   # moving-operand tile (one fp32 PSUM bank)
N_CIC = CIN // P           # input-channel chunks
N_COC = COUT // P          # output-channel chunks
N_WT = W // NT             # output width tiles
WPADDED = W + 2 * PAD
ST = 2 * NT                # output store chunk (overlap tail stores)

F32 = mybir.dt.float32
F32R = mybir.dt.float32r


def _build_program():
    # Bacc (not plain Bass): its finalize() runs generate_event_semaphores,
    # which splits multi-sem waits into event-semaphore chains — the TRN2
    # walrus here accepts at most one sync wait per regular instruction.
    nc = bacc.Bacc()
    # x arrives host-padded: x[b, ci, :] = [0, 0, x_orig, 0, 0] (WPADDED cols)
    x_d = nc.declare_dram_parameter("x", [BPC, CIN, WPADDED], F32, isOutput=False)
    # weights arrive host-transposed in the exact SBUF layout so the load is
    # one DMA of 128 contiguous 10KB lines (a strided load of the same data
    # costs ~12us of per-descriptor processing)
    wt_d = nc.declare_dram_parameter("wt", [P, K, N_CIC, COUT], F32, isOutput=False)
    b_d = nc.declare_dram_parameter("bias2", [P, N_COC], F32, isOutput=False)
    o_d = nc.declare_dram_parameter("out", [BPC, COUT, W], F32, isOutput=True)

    with tile.TileContext(nc) as tc:
        with (
            tc.tile_pool(name="wpool", bufs=1) as wpool,
            tc.tile_pool(name="xpool", bufs=2 * N_CIC) as xpool,
            tc.tile_pool(name="opool", bufs=2 * N_COC) as opool,
            tc.tile_pool(name="psum", bufs=8, space="PSUM") as pspool,
        ):
            # PE warm-up scratch: memset early on DVE, dummy bf16 matmuls
            # below keep the HAM clock-gate busy while x/w stream in, so the
            # real matmul stream starts at 2.4 GHz instead of 1.2 GHz.
            warm = wpool.tile([P, NT], BF16)
            nc.vector.memset(warm[:], 0.0)

            # Weights: one contiguous DMA, issued first:
            # w_sb[ci, k, cic, co_full] = weight[co_full, cic*P+ci, k]
            w_sb = wpool.tile([P, K, N_CIC, COUT], F32R)
            nc.sync.dma_start(w_sb[:], wt_d[:].bitcast(F32R))

            # First batch's x: separate halo tiles (Tile dep tracking is
            # per-tile, so a single chunked tile would gate every matmul on
            # the LAST chunk's DMA). Chunk c holds padded cols
            # [c*2*NT, c*2*NT + 2*NT + 2*PAD) = groups n=2c, 2c+1.
            CWH = 2 * NT + 2 * PAD
            x0c = []  # [cic][c] -> tile
            for cic in range(N_CIC):
                x0c.append([])
                for c in range(N_WT // 2):
                    t = xpool.tile(
                        [P, CWH], F32R, tag="xc", bufs=N_CIC * N_WT // 2,
                        name=f"x0_{cic}_{c}",
                    )
                    x0c[cic].append(t)
            for c in range(N_WT // 2):
                for cic in range(N_CIC):
                    nc.sync.dma_start(
                        x0c[cic][c][:],
                        x_d[0, cic * P:(cic + 1) * P, c * 2 * NT:c * 2 * NT + CWH]
                        .bitcast(F32R),
                    )

            # bias2 host-transposed to [P, N_COC] -> single [128, 2] DMA
            b_sb = wpool.tile([P, N_COC], F32)
            nc.sync.dma_start(b_sb[:], b_d[:])

            ps_warm = pspool.tile([P, NT], F32, tag="ps", name="ps_warm")
            for _ in range(16):
                nc.tensor.matmul(ps_warm[:], warm[:, 0:P], warm[:])

            for b in range(BPC):
                if b > 0:
                    xts = []
                    for cic in range(N_CIC):
                        xt = xpool.tile([P, WPADDED], F32R, tag="x", name=f"x{b}_{cic}")
                        nc.sync.dma_start(
                            xt[:], x_d[b, cic * P:(cic + 1) * P, :].bitcast(F32R)
                        )
                        xts.append(xt)

                last_pass = b == BPC - 1
                for coc in range(N_COC):
                    ot = opool.tile([P, W], F32, tag="o")
                    st = NT if (last_pass and coc == N_COC - 1) else ST
                    for n in range(N_WT):
                        ps = pspool.tile([P, NT], F32, tag="ps", name=f"ps{b}_{coc}_{n}")
                        idx = 0
                        for k in range(K):
                            for cic in range(N_CIC):
                                if b == 0:
                                    rhs = x0c[cic][n // 2][
                                        :, (n % 2) * NT + k:(n % 2) * NT + k + NT
                                    ]
                                else:
                                    rhs = xts[cic][:, n * NT + k:n * NT + k + NT]
                                nc.tensor.matmul(
                                    ps[:],
                                    w_sb[:, k, cic, coc * P:(coc + 1) * P],
                                    rhs,
                                    start=(idx == 0),
                                    stop=(idx == K * N_CIC - 1),
                                )
                                idx += 1
                        nc.vector.tensor_scalar_add(
                            ot[:, n * NT:(n + 1) * NT], ps[:], b_sb[:, coc:coc + 1]
                        )
                        # store as soon as a full chunk of st cols is ready
                        if ((n + 1) * NT) % st == 0:
                            c0 = (n + 1) * NT - st
                            nc.sync.dma_start(
                                o_d[b, coc * P:(coc + 1) * P, c0:c0 + st],
                                ot[:, c0:c0 + st],
                            )
    nc.finalize()
    return nc


_NC_CACHE = []


def kernel(x, weight, bias):
    assert x.shape == (B, CIN, W) and weight.shape == (COUT, CIN, K)
    if not _NC_CACHE:
        _NC_CACHE.append(_build_program())
    nc = _NC_CACHE[0]

    # wt[ci, k, cic, co] = weight[co, cic*128+ci, k]  (SBUF layout)
    wt = np.ascontiguousarray(
        weight.astype(np.float32)
        .transpose(1, 2, 0)               # [ci_full, k, co]
        .reshape(N_CIC, P, K, COUT)       # [cic, ci, k, co]
        .transpose(1, 2, 0, 3)            # [ci, k, cic, co]
    )
    bias2 = np.ascontiguousarray(bias.astype(np.float32).reshape(N_COC, P).T)
    xpad = np.pad(x.astype(np.float32), ((0, 0), (0, 0), (PAD, PAD)))
    in_maps = [
        {
            "x": np.ascontiguousarray(xpad[i * BPC:(i + 1) * BPC]),
            "wt": wt,
            "bias2": bias2,
        }
        for i in range(NCORES)
    ]
    res = run_bass_kernel_spmd(
        nc,
        in_maps,
        list(range(NCORES)),
        trace=bool(int(os.environ.get("KERNEL_TRACE", "0"))),
    )
    kernel.last_results = res
    return np.concatenate([res.results[i]["out"] for i in range(NCORES)], axis=0)



# revision 4
# speedup vs baseline: 1.0401x; 1.0401x over previous
"""Trainium2 Bass kernel for CustomConv1d.

Problem: y = conv1d(x, weight, bias), x [32, 256, 4096] f32,
weight [256, 256, 5] f32, bias [256] f32, stride 1, pad 2.

Strategy: data-parallel over batch across 8 NeuronCores (4 batches/core,
weights+bias broadcast, no collectives). Per core the conv is computed as
matmuls on the tensor engine: for each output-channel chunk (128) and each
512-wide output tile, accumulate 10 matmuls in PSUM (5 taps x 2 input-channel
chunks of 128):

  out[co, w] = sum_{k, ci} weight[co, ci, k] * xpad[ci, w + k]

with lhsT = weight slice [ci(128 part), co(128)] and rhs = x slice
[ci(128 part), 512]. Operands are cast to bf16 on the host (~2.3e-3 l2 rel
err, well under the 2e-2 gate); PSUM accumulates fp32 and the output is
stored fp32. bf16 halves the x/weight HBM traffic and avoids the fp32-HIGH
PE power-throttle. x arrives host-padded ([.., W+4]) so every tap is a plain
contiguous slice.

Startup is DMA-latency critical: the weight load is triggered first, then
batch 0's first x chunk, so the first real matmul isn't gated on the full
batch-0 load. Batches 1-3's x loads are triggered from the Vector engine
mid-stream (after an early bias-add of the previous batch) so they don't
steal HBM bandwidth from the startup-critical loads. Output stores trigger
from the Scalar engine (otherwise idle) to keep the Sync queue short.
"""

import os

import numpy as np
import ml_dtypes

import concourse.mybir as mybir
import concourse.tile as tile
from concourse import bacc
from concourse.bass_utils import run_bass_kernel_spmd


BF16 = mybir.dt.bfloat16
F32 = mybir.dt.float32

B, CIN, COUT, W, K, PAD = 32, 256, 256, 4096, 5, 2
NCORES = 8
BPC = B // NCORES          # batches per core
P = 128                    # partition dim
NT = 512                   # moving-operand tile (one fp32 PSUM bank)
N_CIC = CIN // P           # input-channel chunks
N_COC = COUT // P          # output-channel chunks
N_WT = W // NT             # output width tiles
WPADDED = W + 2 * PAD
ST = 2 * NT                # output store chunk (overlap tail stores)
NWARM = 10                 # PE warm-up matmuls (cover DMA-start latency)


def _build_program():
    # Bacc (not plain Bass): its finalize() runs generate_event_semaphores,
    # which splits multi-sem waits into event-semaphore chains — the TRN2
    # walrus here accepts at most one sync wait per regular instruction.
    nc = bacc.Bacc()
    # x arrives host-padded: x[b, ci, :] = [0, 0, x_orig, 0, 0] (WPADDED cols)
    x_d = nc.declare_dram_parameter("x", [BPC, CIN, WPADDED], BF16, isOutput=False)
    # weights arrive host-transposed in the exact SBUF layout so the load is
    # one DMA of 128 contiguous lines
    wt_d = nc.declare_dram_parameter("wt", [P, K, N_CIC, COUT], BF16, isOutput=False)
    b_d = nc.declare_dram_parameter("bias2", [P, N_COC], F32, isOutput=False)
    o_d = nc.declare_dram_parameter("out", [BPC, COUT, W], F32, isOutput=True)

    with tile.TileContext(nc) as tc:
        with (
            tc.tile_pool(name="wpool", bufs=1) as wpool,
            tc.tile_pool(name="xpool", bufs=2 * N_CIC) as xpool,
            tc.tile_pool(name="opool", bufs=2 * N_COC) as opool,
            tc.tile_pool(name="psum", bufs=8, space="PSUM") as pspool,
        ):
            # PE warm-up scratch: memset early on DVE, dummy bf16 matmuls
            # below keep the HAM clock-gate busy while x/w stream in, so the
            # real matmul stream starts at 2.4 GHz instead of 1.2 GHz.
            warm = wpool.tile([P, NT], BF16)
            nc.vector.memset(warm[:], 0.0)

            # Weights first: they gate the very first LDWEIGHTS.
            w_sb = wpool.tile([P, K, N_CIC, COUT], BF16)
            nc.sync.dma_start(w_sb[:], wt_d[:])

            # First batch's x: separate halo tiles (Tile dep tracking is
            # per-tile, so a single chunked tile would gate every matmul on
            # the LAST chunk's DMA). Chunk c holds padded cols
            # [c*2*NT, c*2*NT + 2*NT + 2*PAD) = groups n=2c, 2c+1.
            CWH = 2 * NT + 2 * PAD
            x0c = []  # [cic][c] -> tile
            for cic in range(N_CIC):
                x0c.append([])
                for c in range(N_WT // 2):
                    t = xpool.tile(
                        [P, CWH], BF16, tag="xc", bufs=N_CIC * N_WT // 2,
                        name=f"x0_{cic}_{c}",
                    )
                    x0c[cic].append(t)
            # chunk 0 right after the weights (both gate the first matmul)...
            for cic in range(N_CIC):
                nc.sync.dma_start(
                    x0c[cic][0][:], x_d[0, cic * P:(cic + 1) * P, 0:CWH]
                )
            # bias2 host-transposed to [P, N_COC] -> single [128, 2] DMA
            b_sb = wpool.tile([P, N_COC], F32)
            nc.sync.dma_start(b_sb[:], b_d[:])
            # ... then the rest of batch 0.
            for c in range(1, N_WT // 2):
                for cic in range(N_CIC):
                    nc.sync.dma_start(
                        x0c[cic][c][:],
                        x_d[0, cic * P:(cic + 1) * P,
                            c * 2 * NT:c * 2 * NT + CWH],
                    )

            ps_warm = pspool.tile([P, NT], F32, tag="ps", name="ps_warm")
            for _ in range(NWARM):
                nc.tensor.matmul(ps_warm[:], warm[:, 0:P], warm[:])

            xts_by_b = {}
            for b in range(BPC):
                last_pass = b == BPC - 1
                for coc in range(N_COC):
                    ot = opool.tile([P, W], F32, tag="o")
                    st = NT if (last_pass and coc == N_COC - 1) else ST
                    for n in range(N_WT):
                        ps = pspool.tile([P, NT], F32, tag="ps", name=f"ps{b}_{coc}_{n}")
                        idx = 0
                        for k in range(K):
                            for cic in range(N_CIC):
                                if b == 0:
                                    rhs = x0c[cic][n // 2][
                                        :, (n % 2) * NT + k:(n % 2) * NT + k + NT
                                    ]
                                else:
                                    rhs = xts_by_b[b][cic][:, n * NT + k:n * NT + k + NT]
                                nc.tensor.matmul(
                                    ps[:],
                                    w_sb[:, k, cic, coc * P:(coc + 1) * P],
                                    rhs,
                                    start=(idx == 0),
                                    stop=(idx == K * N_CIC - 1),
                                )
                                idx += 1
                        nc.vector.tensor_scalar_add(
                            ot[:, n * NT:(n + 1) * NT], ps[:], b_sb[:, coc:coc + 1]
                        )
                        # store as soon as a full chunk of st cols is ready
                        # (trigger from the otherwise-idle Scalar engine)
                        if ((n + 1) * NT) % st == 0:
                            c0 = (n + 1) * NT - st
                            nc.scalar.dma_start(
                                o_d[b, coc * P:(coc + 1) * P, c0:c0 + st],
                                ot[:, c0:c0 + st],
                            )
                        # Next batch's x load triggers from the Scalar queue
                        # right after the 2nd store trigger of the previous
                        # batch (its queue position is the timing anchor): it
                        # neither competes with startup-critical DMA (t=0)
                        # nor arrives late for batch b+1's first matmul.
                        if coc == 0 and n == 3 and b + 1 < BPC:
                            xts = []
                            for cic in range(N_CIC):
                                xt = xpool.tile(
                                    [P, WPADDED], BF16, tag="x",
                                    name=f"x{b + 1}_{cic}",
                                )
                                nc.scalar.dma_start(
                                    xt[:], x_d[b + 1, cic * P:(cic + 1) * P, :]
                                )
                                xts.append(xt)
                            xts_by_b[b + 1] = xts
    nc.finalize()
    return nc


_NC_CACHE = []


def kernel(x, weight, bias):
    assert x.shape == (B, CIN, W) and weight.shape == (COUT, CIN, K)
    if not _NC_CACHE:
        _NC_CACHE.append(_build_program())
    nc = _NC_CACHE[0]

    # wt[ci, k, cic, co] = weight[co, cic*128+ci, k]  (SBUF layout)
    wt = np.ascontiguousarray(
        weight.astype(np.float32)
        .transpose(1, 2, 0)               # [ci_full, k, co]
        .reshape(N_CIC, P, K, COUT)       # [cic, ci, k, co]
        .transpose(1, 2, 0, 3)            # [ci, k, cic, co]
    ).astype(ml_dtypes.bfloat16)
    bias2 = np.ascontiguousarray(bias.astype(np.float32).reshape(N_COC, P).T)
    xpad = np.pad(x.astype(np.float32), ((0, 0), (0, 0), (PAD, PAD))).astype(
        ml_dtypes.bfloat16
    )
    in_maps = [
        {
            "x": np.ascontiguousarray(xpad[i * BPC:(i + 1) * BPC]),
            "wt": wt,
            "bias2": bias2,
        }
        for i in range(NCORES)
    ]
    res = run_bass_kernel_spmd(
        nc,
        in_maps,
        list(range(NCORES)),
        trace=bool(int(os.environ.get("KERNEL_TRACE", "0"))),
    )
    kernel.last_results = res
    return np.concatenate([res.results[i]["out"] for i in range(NCORES)], axis=0)


# revision 5
# speedup vs baseline: 1.0717x; 1.0304x over previous
"""Trainium2 Bass kernel for CustomConv1d.

Problem: y = conv1d(x, weight, bias), x [32, 256, 4096] f32,
weight [256, 256, 5] f32, bias [256] f32, stride 1, pad 2.

Strategy: data-parallel over batch across 8 NeuronCores (4 batches/core,
weights+bias broadcast, no collectives). Per core the conv is computed as
matmuls on the tensor engine: for each output-channel chunk (128) and each
512-wide output tile, accumulate 10 matmuls in PSUM (5 taps x 2 input-channel
chunks of 128):

  out[co, w] = sum_{k, ci} weight[co, ci, k] * xpad[ci, w + k]

with lhsT = weight slice [ci(128 part), co(128)] and rhs = x slice
[ci(128 part), 512]. Operands are cast to bf16 on the host (~2.3e-3 l2 rel
err, well under the 2e-2 gate); PSUM accumulates fp32 and the output is
stored fp32. bf16 halves the x/weight HBM traffic and avoids the fp32-HIGH
PE power-throttle (fp32 matmul cadence is ~233 ns/512 rows vs 216 for bf16).
x arrives host-padded ([.., W+4]) so every tap is a plain contiguous slice.

Startup is DMA-latency critical (engine preambles end ~6-7 us; the PE floor
is ~138 us of back-to-back matmuls, so every us the first matmul is delayed
is lost). Measures taken:
 - weights live in 3 per-k-slice tiles so the first matmul gates only on
   the first 131 KB slice, not the full load;
 - their triggers go on the Scalar queue, which exits its preamble ~1 us
   before Sync;
 - batch 0's first two width-groups use small dedicated x tiles (132 KB)
   interleaved with the weight slices;
 - all remaining x arrives as uniform halo-chunk tiles from one pool tag
   (bufs=14), whose buffer reuse (WAR on chunks consumed mid-previous-batch)
   is what defers later batches' loads: the Tile scheduler is a greedy list
   scheduler and hoists anything without real dependencies to t=0, where it
   would steal HBM bandwidth from the startup-critical loads.

The tail is trimmed by splitting the last group's bias-add + store into
256-col pieces so the final store isn't waiting on a full 512-col drain.
"""

import os

import numpy as np
import ml_dtypes

import concourse.mybir as mybir
import concourse.tile as tile
from concourse import bacc
from concourse.bass_utils import run_bass_kernel_spmd


BF16 = mybir.dt.bfloat16
F32 = mybir.dt.float32

B, CIN, COUT, W, K, PAD = 32, 256, 256, 4096, 5, 2
NCORES = 8
BPC = B // NCORES          # batches per core
P = 128                    # partition dim
NT = 512                   # moving-operand tile (one fp32 PSUM bank)
N_CIC = CIN // P           # input-channel chunks
N_COC = COUT // P          # output-channel chunks
N_WT = W // NT             # output width tiles
WPADDED = W + 2 * PAD
ST = 2 * NT                # output store chunk (overlap tail stores)
CWH = 2 * NT + 2 * PAD     # halo chunk: 2 groups + taps
NTF = NT + 2 * PAD         # fine first-chunk: 1 group + taps
NWARM = 2                  # PE warm-up matmuls (bridge preamble->first data)
NB_XC = 14                 # x chunk pool depth (sets the prefetch lag)
# weight k-slices: first 131 KB slice alone gates the first matmul
K_SLICES = [(0, 1), (1, 3), (3, 5)]


def _build_program():
    # Bacc (not plain Bass): its finalize() runs generate_event_semaphores,
    # which splits multi-sem waits into event-semaphore chains — the TRN2
    # walrus here accepts at most one sync wait per regular instruction.
    nc = bacc.Bacc()
    # x arrives host-padded: x[b, ci, :] = [0, 0, x_orig, 0, 0] (WPADDED cols)
    x_d = nc.declare_dram_parameter("x", [BPC, CIN, WPADDED], BF16, isOutput=False)
    # weights arrive host-transposed in the exact SBUF layout so each k-slice
    # load is one DMA of 128 contiguous lines
    wt_d = nc.declare_dram_parameter("wt", [P, K, N_CIC, COUT], BF16, isOutput=False)
    b_d = nc.declare_dram_parameter("bias2", [P, N_COC], F32, isOutput=False)
    o_d = nc.declare_dram_parameter("out", [BPC, COUT, W], F32, isOutput=True)

    with tile.TileContext(nc) as tc:
        with (
            tc.tile_pool(name="wpool", bufs=1) as wpool,
            tc.tile_pool(name="xpool", bufs=NB_XC) as xpool,
            tc.tile_pool(name="opool", bufs=2 * N_COC) as opool,
            tc.tile_pool(name="psum", bufs=8, space="PSUM") as pspool,
        ):
            warm = wpool.tile([P, NT], BF16)
            nc.vector.memset(warm[:], 0.0)

            # Weight slices + batch 0's first two groups, interleaved on the
            # Scalar trigger queue (first out of preamble).
            w_k = []  # w_k[i] covers taps K_SLICES[i]
            for i, (k0, k1) in enumerate(K_SLICES):
                w_k.append(wpool.tile([P, k1 - k0, N_CIC, COUT], BF16,
                                      name=f"w_k{i}"))
            xf = [[None, None], [None, None]]  # [g][cic]
            for g in range(2):
                for cic in range(N_CIC):
                    xf[g][cic] = xpool.tile(
                        [P, NTF], BF16, tag="xf", bufs=4, name=f"xf{g}_{cic}"
                    )
            nc.scalar.dma_start(w_k[0][:], wt_d[:, 0:1])
            for cic in range(N_CIC):
                nc.scalar.dma_start(
                    xf[0][cic][:], x_d[0, cic * P:(cic + 1) * P, 0:NTF]
                )
            nc.scalar.dma_start(w_k[1][:], wt_d[:, 1:3])
            for cic in range(N_CIC):
                nc.scalar.dma_start(
                    xf[1][cic][:], x_d[0, cic * P:(cic + 1) * P, NT:NT + NTF]
                )
            nc.scalar.dma_start(w_k[2][:], wt_d[:, 3:5])
            b_sb = wpool.tile([P, N_COC], F32)
            nc.scalar.dma_start(b_sb[:], b_d[:])

            # All remaining x: uniform halo chunks from one tag. Allocation
            # order (b0 c1..c7, then b1..b3 c0..c7) + bufs=NB_XC gives each
            # chunk a WAR dependency on the chunk consumed ~7 pairs earlier,
            # which is both the prefetch schedule and the startup-bandwidth
            # guard. Chunk (b, c) covers padded cols [1024c, 1024c + CWH).
            xc = {}  # (b, c, cic) -> tile
            for b in range(BPC):
                for c in range(0 if b else 1, N_WT // 2):
                    for cic in range(N_CIC):
                        t = xpool.tile(
                            [P, CWH], BF16, tag="xc", bufs=NB_XC,
                            name=f"x{b}_{c}_{cic}",
                        )
                        xc[(b, c, cic)] = t
                        nc.sync.dma_start(
                            t[:],
                            x_d[b, cic * P:(cic + 1) * P,
                                c * 2 * NT:c * 2 * NT + CWH],
                        )

            ps_warm = pspool.tile([P, NT], F32, tag="ps", name="ps_warm")
            for _ in range(NWARM):
                nc.tensor.matmul(ps_warm[:], warm[:, 0:P], warm[:])

            for b in range(BPC):
                last_pass = b == BPC - 1
                for coc in range(N_COC):
                    last_coc = last_pass and coc == N_COC - 1
                    ot = opool.tile([P, W], F32, tag="o")
                    st = NT if last_coc else ST
                    for n in range(N_WT):
                        ps = pspool.tile([P, NT], F32, tag="ps", name=f"ps{b}_{coc}_{n}")
                        idx = 0
                        for k in range(K):
                            ws, (k0, _) = next(
                                (w_k[i], K_SLICES[i])
                                for i in range(len(K_SLICES))
                                if K_SLICES[i][0] <= k < K_SLICES[i][1]
                            )
                            for cic in range(N_CIC):
                                if b == 0 and n < 2:
                                    rhs = xf[n][cic][:, k:k + NT]
                                else:
                                    rhs = xc[(b, n // 2, cic)][
                                        :, (n % 2) * NT + k:(n % 2) * NT + k + NT
                                    ]
                                nc.tensor.matmul(
                                    ps[:],
                                    ws[:, k - k0, cic, coc * P:(coc + 1) * P],
                                    rhs,
                                    start=(idx == 0),
                                    stop=(idx == K * N_CIC - 1),
                                )
                                idx += 1
                        if last_coc and n == N_WT - 1:
                            # final group: drain + store in 256-col pieces so
                            # the kernel's last store waits on a quarter-size
                            # bias-add and moves half the bytes
                            for h in range(2):
                                lo = n * NT + h * (NT // 2)
                                nc.vector.tensor_scalar_add(
                                    ot[:, lo:lo + NT // 2],
                                    ps[:, h * (NT // 2):(h + 1) * (NT // 2)],
                                    b_sb[:, coc:coc + 1],
                                )
                                nc.scalar.dma_start(
                                    o_d[b, coc * P:(coc + 1) * P, lo:lo + NT // 2],
                                    ot[:, lo:lo + NT // 2],
                                )
                        else:
                            nc.vector.tensor_scalar_add(
                                ot[:, n * NT:(n + 1) * NT], ps[:],
                                b_sb[:, coc:coc + 1],
                            )
                            # store as soon as a full chunk of st cols is
                            # ready (trigger from the Scalar engine; Sync is
                            # busy pacing the x chunk triggers)
                            if ((n + 1) * NT) % st == 0:
                                c0 = (n + 1) * NT - st
                                nc.scalar.dma_start(
                                    o_d[b, coc * P:(coc + 1) * P, c0:c0 + st],
                                    ot[:, c0:c0 + st],
                                )
    nc.finalize()
    return nc


_NC_CACHE = []


def kernel(x, weight, bias):
    assert x.shape == (B, CIN, W) and weight.shape == (COUT, CIN, K)
    if not _NC_CACHE:
        _NC_CACHE.append(_build_program())
    nc = _NC_CACHE[0]

    # wt[ci, k, cic, co] = weight[co, cic*128+ci, k]  (SBUF layout)
    wt = np.ascontiguousarray(
        weight.astype(np.float32)
        .transpose(1, 2, 0)               # [ci_full, k, co]
        .reshape(N_CIC, P, K, COUT)       # [cic, ci, k, co]
        .transpose(1, 2, 0, 3)            # [ci, k, cic, co]
    ).astype(ml_dtypes.bfloat16)
    bias2 = np.ascontiguousarray(bias.astype(np.float32).reshape(N_COC, P).T)
    xpad = np.pad(x.astype(np.float32), ((0, 0), (0, 0), (PAD, PAD))).astype(
        ml_dtypes.bfloat16
    )
    in_maps = [
        {
            "x": np.ascontiguousarray(xpad[i * BPC:(i + 1) * BPC]),
            "wt": wt,
            "bias2": bias2,
        }
        for i in range(NCORES)
    ]
    res = run_bass_kernel_spmd(
        nc,
        in_maps,
        list(range(NCORES)),
        trace=bool(int(os.environ.get("KERNEL_TRACE", "0"))),
    )
    kernel.last_results = res
    return np.concatenate([res.results[i]["out"] for i in range(NCORES)], axis=0)
